# revision 1
# baseline (speedup 1.0000x reference)
"""Batch semi-hard triplet loss (cosine distance) on 8 Trainium2 NeuronCores.

Strategy (data-parallel over rows, per sharding hint):
  - Host: sort rows by label; core c takes sorted rows [1024c, 1024(c+1)) in
    8 exact 128-row M-tiles (classes may straddle tile/core boundaries -- the
    per-row class-range poison masks handle any split).  Columns are rotated
    per core so its rows' class columns sit near column 0 (1-2 diag N-tiles
    per M-tile).
  - Device (per core, uniform SPMD program):
      * normalize embeddings: squares (DVE/Act split), one-hot accumulating
        matmuls for column norms, sqrt (Act) + reciprocal (DVE), one-hot
        broadcast matmuls, column scale (DVE) -- two-stage pipelined chains;
      * prologue (all M-tiles up front, so the main loop never ping-pongs
        between engines): diag-bank matmuls, wm = dot + pois(-2 on class
        cols) into a persistent buffer, positive-side min over the narrow
        class-column window -> t_p, ntp;
      * main loop per M-tile: 16 matmuls (4-bank PSUM groups) -> Act streams
        u = 1/(dot - t_p) (ScalarE reciprocal, per-partition bias; diag banks
        read wm) at 2048 wide; DVE min-tree over u -> r1 = min u, one tile
        behind.
  - Host: q = t_p + 1/r1 (largest dot strictly below t_p); per-row loss
    epilogue in f64; rows without a semi-hard negative in the margin window
    (or near the branch boundary) are recomputed exactly in f32 numpy; mean
    over valid rows.
"""

import numpy as np
import ml_dtypes

B = 8192
D = 128
MARGIN = 0.2
NCORES = 8
NT = 512            # N-tile width (one PSUM bank of fp32)
N_NT = B // NT      # 16
MT = 128            # M-tile rows
NMT = B // NCORES // MT  # 8 m-tiles per core
GW = 4              # N-tiles per PSUM group tile
POIS = -2.0         # class-column poison (exactly representable in bf16)

BF16 = ml_dtypes.bfloat16

_CACHE = {}


# --------------------------------------------------------------------------
# host-side planning (pure layout, computed from labels)
# --------------------------------------------------------------------------
def _plan(labels: np.ndarray):
    order = np.argsort(labels, kind="stable")
    slab = labels[order]
    bounds = np.flatnonzero(np.r_[True, slab[1:] != slab[:-1], True])
    cls_start, cls_end = bounds[:-1], bounds[1:]
    # per sorted row: its class range [s, e)
    row_s = np.empty(B, dtype=np.int64)
    row_e = np.empty(B, dtype=np.int64)
    for s, e in zip(cls_start, cls_end):
        row_s[s:e] = s
        row_e[s:e] = e

    rows_per_core = B // NCORES
    cores = []
    for c in range(NCORES):
        r0 = c * rows_per_core
        base = int(row_s[r0])  # start of first class -> no wraparound
        # per m-tile: diag N-tiles touched by its rows' class ranges (rotated)
        diag = []
        for m in range(NMT):
            rr = slice(r0 + m * MT, r0 + (m + 1) * MT)
            s = row_s[rr] - base
            e = row_e[rr] - base
            dts = sorted(set((s // NT).tolist()) | set(((e - 1) // NT).tolist()))
            diag.append(dts)
        cores.append(dict(r0=r0, base=base, diag=diag))
    # unify diag sets across cores so all 8 run one compiled program
    uni = [
        sorted(set().union(*[set(pc["diag"][m]) for pc in cores]))
        for m in range(NMT)
    ]
    for pc in cores:
        pc["diag"] = uni
    # per (m, diag tile): narrow column window [c0, c1) within the bank that
    # contains every class column of the tile's rows, across all cores (the
    # positive-side min may be restricted to it: non-class dots can't win)
    wins = []
    for m in range(NMT):
        wm_ = []
        for d in uni[m]:
            c0, c1 = NT, 0
            for pc in cores:
                rr = slice(pc["r0"] + m * MT, pc["r0"] + (m + 1) * MT)
                s = np.maximum(row_s[rr] - pc["base"] - d * NT, 0)
                e = np.minimum(row_e[rr] - pc["base"] - d * NT, NT)
                ok = s < e
                if ok.any():
                    c0 = min(c0, int(s[ok].min()))
                    c1 = max(c1, int(e[ok].max()))
            if c1 <= c0:
                c0, c1 = 0, NT
            wm_.append((c0, c1))
        wins.append(wm_)
    return dict(
        order=order, row_s=row_s, row_e=row_e, cores=cores, diag=uni, wins=wins
    )


def _build_core_inputs(emb_sorted: np.ndarray, plan, c: int):
    """Returns (xt_rot [D,B] bf16, xb [D,1024] bf16, mk [128, nblk*NT] bf16,
    oh [128, NOH*NOH] bf16)."""
    pc = plan["cores"][c]
    base, r0 = pc["base"], pc["r0"]
    rows_per_core = B // NCORES

    rot = np.r_[np.arange(base, B), np.arange(0, base)]
    xt_rot = np.ascontiguousarray(emb_sorted[rot].T).astype(BF16)
    xb = np.ascontiguousarray(emb_sorted[r0 : r0 + rows_per_core].T).astype(BF16)

    # poison tiles: per (m, d in diag[m]) a [128, NT] block, -2 on class cols
    nblk = sum(len(d) for d in pc["diag"])
    mk = np.zeros((MT, nblk * NT), np.float32)
    bi = 0
    for m in range(NMT):
        for d in pc["diag"][m]:
            for r in range(MT):
                g = r0 + m * MT + r
                s = int(plan["row_s"][g]) - base - d * NT
                e = int(plan["row_e"][g]) - base - d * NT
                s, e = max(s, 0), min(e, NT)
                if s < e:
                    mk[r, bi * NT + s : bi * NT + e] = POIS
            bi += 1
    mk = mk.astype(BF16)

    # one-hot lhsT blocks for the n2 accumulating matmuls: tile t's column
    # sums land on psum row t % 4 (per-4-tile-group chains at partition 0)
    NOH = N_NT + 2  # 16 xt tiles + 2 xb tiles
    oh = np.zeros((D, 4 * NOH), np.float32)
    for t in range(NOH):
        oh[:, 4 * t + (t % 4)] = 1.0
    oh = oh.astype(BF16)
    # one-hot lhsT blocks for the rn broadcast matmuls:
    # ob[k, D*t + i] = 1 iff k == t % 4  ->  out[i, j] = rn_grp[t % 4, j]
    ob = np.zeros((4, D * NOH), np.float32)
    for t in range(NOH):
        ob[t % 4, D * t : D * (t + 1)] = 1.0
    ob = ob.astype(BF16)
    return xt_rot, xb, mk, oh, ob


# --------------------------------------------------------------------------
# device program
# --------------------------------------------------------------------------
def _raw_recip_bias(nc, out, in_, bias_ap):
    import concourse.mybir as mybir

    eng = nc.scalar
    ins = [
        eng.lower_ap(in_),
        eng.lower_ap(bias_ap),
        mybir.ImmediateValue(dtype=mybir.dt.float32, value=1.0),  # scale
        mybir.ImmediateValue(dtype=mybir.dt.float32, value=0.0),  # alpha
    ]
    return eng.add_instruction(
        mybir.InstActivation(
            name=f"I-{nc.next_id()}",
            func=mybir.ActivationFunctionType.Reciprocal,
            ins=ins,
            outs=[eng.lower_ap(out)],
        )
    )


def _build_bass(diag, wins, mask_k: int):
    import concourse.bacc as bacc
    import concourse.mybir as mybir
    from concourse.tile import TileContext

    f32 = mybir.dt.float32
    bf16 = mybir.dt.bfloat16
    Alu = mybir.AluOpType
    Act = mybir.ActivationFunctionType
    FMAX = 3.0e38
    NOH = N_NT + 2
    NBC = NMT * MT  # xb columns (1024)

    nc = bacc.Bacc("TRN2", target_bir_lowering=False, debug=False, num_devices=NCORES)

    xt_d = nc.dram_tensor("xt", [D, B], bf16, kind="ExternalInput").ap()
    xb_d = nc.dram_tensor("xb", [D, NBC], bf16, kind="ExternalInput").ap()
    mk_d = nc.dram_tensor("mk", [MT, mask_k], bf16, kind="ExternalInput").ap()
    oh_d = nc.dram_tensor("oh", [D, 4 * NOH], bf16, kind="ExternalInput").ap()
    ob_d = nc.dram_tensor("ob", [4, D * NOH], bf16, kind="ExternalInput").ap()
    out_d = nc.dram_tensor("out", [MT, 2 * NMT], f32, kind="ExternalOutput").ap()

    with TileContext(nc) as tc:
        with (
            tc.tile_pool(name="big", bufs=1) as big,
            tc.tile_pool(name="upool", bufs=2) as upool,
            tc.tile_pool(name="scr", bufs=2) as scr,
            tc.tile_pool(name="wmp", bufs=6) as wmp,
            tc.tile_pool(name="sm", bufs=6) as smp,
            tc.tile_pool(name="psw", bufs=8 // GW, space="PSUM") as psw,
        ):
            # ---------------- setup: load + normalize (pipelined) -----------
            # small tensors first so nothing downstream waits on them
            oh = big.tile([D, 4 * NOH], bf16, tag="oh")
            nc.sync.dma_start(oh[:], oh_d)
            ob = big.tile([4, D * NOH], bf16, tag="ob")
            nc.sync.dma_start(ob[:], ob_d)
            xb = big.tile([D, NBC], bf16, tag="xb")
            nc.sync.dma_start(xb[:], xb_d)
            mk = big.tile([MT, mask_k], bf16, tag="mk")
            nchunk = max(1, mask_k // 4096)
            cw = (mask_k + nchunk - 1) // nchunk
            for j in range(nchunk):
                sl = slice(j * cw, min((j + 1) * cw, mask_k))
                nc.sync.dma_start(mk[:, sl], mk_d[:, sl])
            xt = big.tile([D, B], bf16, tag="xt")
            for j in range(8):
                sl = slice(j * (B // 8), (j + 1) * (B // 8))
                nc.sync.dma_start(xt[:, sl], xt_d[:, sl])

            sq = big.tile([D, NOH * NT], bf16, tag="sq")
            xtn = big.tile([D, B], bf16, tag="xtn")
            xbn = big.tile([D, NBC], bf16, tag="xbn")
            outb = big.tile([MT, 2 * NMT], f32, tag="outb")

            def norm_stage_a(nrows, tbase, src, act_sq):
                """squares + one-hot n2 matmuls + psum->sbuf copy + sqrt"""
                w = nrows * NT
                if act_sq:
                    nc.scalar.activation(sq[:, tbase * NT : tbase * NT + w],
                                         src, Act.Square)
                else:
                    nc.vector.tensor_tensor(sq[:, tbase * NT : tbase * NT + w],
                                            src, src, Alu.mult)
                pn = psw.tile([MT, GW * NT], f32, tag="w", name="pn")
                for k in range(nrows):
                    t = tbase + k
                    nc.tensor.matmul(
                        pn[0:4, :NT], oh[:, 4 * t : 4 * (t + 1)],
                        sq[:, t * NT : (t + 1) * NT],
                        start=(k == 0), stop=(k == nrows - 1),
                    )
                n2g = smp.tile([4, NT], f32, tag="n2g", name="n2g", bufs=3)
                nc.scalar.copy(n2g[0:nrows, :], pn[0:nrows, :NT])
                s0g = smp.tile([4, NT], f32, tag="s0g", name="s0g", bufs=3)
                nc.scalar.activation(s0g[0:nrows, :], n2g[0:nrows, :], Act.Sqrt)
                return s0g

            def norm_stage_b(s0g, nrows, tbase, src, dst):
                """reciprocal -> bf16 rn -> broadcast matmuls -> scaled dst"""
                w = nrows * NT
                r0g = smp.tile([4, NT], f32, tag="r0g", name="r0g", bufs=3)
                nc.vector.reciprocal(r0g[0:nrows, :], s0g[0:nrows, :])
                rng_ = smp.tile([4, NT], bf16, tag="rng", name="rng", bufs=3)
                nc.scalar.copy(rng_[0:nrows, :], r0g[0:nrows, :])
                gp = psw.tile([MT, GW * NT], f32, tag="w", name="gp")
                for k in range(nrows):
                    t = tbase + k
                    nc.tensor.matmul(
                        gp[:, k * NT : (k + 1) * NT],
                        ob[0:nrows, D * t : D * (t + 1)], rng_[0:nrows, :],
                    )
                nc.vector.tensor_tensor(dst, src, gp[:, :w], Alu.mult)

            # two-stage pipelined chains: xb first (xbn gates the prologue)
            chains = [
                (2, N_NT, xb[:], xbn[:], False),
            ] + [
                (4, 4 * gq, xt[:, 4 * gq * NT : (4 * gq + 4) * NT],
                 xtn[:, 4 * gq * NT : (4 * gq + 4) * NT], gq >= 2)
                for gq in range(4)
            ]
            pend = None
            for ch in chains:
                nrows, tbase, src, dst, act_sq = ch
                s0g = norm_stage_a(nrows, tbase, src, act_sq)
                if pend is not None:
                    norm_stage_b(*pend)
                pend = (s0g, nrows, tbase, src, dst)
            norm_stage_b(*pend)

            # ---------------- prologue: t_p per M-tile ----------------
            # diag-bank matmuls (packed 4 per PSUM tile), wm = dot + pois into
            # a persistent buffer, narrow positive-min reduce -> t_p, ntp
            nblk = sum(len(d) for d in diag)
            wmbig = big.tile([MT, nblk * NT], f32, tag="wmbig")
            ntpall = big.tile([MT, NMT], f32, tag="ntpall")

            flat = []  # (m, j, d) in diag-block order
            for m in range(NMT):
                for j, d in enumerate(diag[m]):
                    flat.append((m, j, d))
            posms = {}
            for m in range(NMT):
                posms[m] = smp.tile(
                    [MT, max(len(diag[m]), 1)], f32, tag=f"posm{m}",
                    name=f"posm{m}",
                )
            bi = 0
            while bi < nblk:
                hi = min(bi + 4, nblk)
                pg = psw.tile([MT, GW * NT], f32, tag="w")
                for i in range(bi, hi):
                    m, j, d = flat[i]
                    nc.tensor.matmul(
                        pg[:, (i - bi) * NT : (i - bi + 1) * NT],
                        xbn[:, m * MT : (m + 1) * MT],
                        xtn[:, d * NT : (d + 1) * NT],
                    )
                for i in range(bi, hi):
                    m, j, d = flat[i]
                    c0, c1 = wins[m][j]
                    # narrow poison-add: only the class-column window matters
                    # for the positive-side min (non-class dots can't win)
                    nc.vector.tensor_tensor(
                        wmbig[:, i * NT + c0 : i * NT + c1],
                        pg[:, (i - bi) * NT + c0 : (i - bi) * NT + c1],
                        mk[:, i * NT + c0 : i * NT + c1], Alu.add,
                    )
                    nc.vector.tensor_reduce(
                        posms[m][:, j : j + 1],
                        wmbig[:, i * NT + c0 : i * NT + c1],
                        axis=mybir.AxisListType.X, op=Alu.min,
                    )
                    if j == len(diag[m]) - 1:
                        ndts = len(diag[m])
                        if ndts == 1:
                            minpos = posms[m][:, 0:1]
                        elif ndts == 2:
                            minpos = smp.tile([MT, 1], f32, tag="minpos")
                            nc.vector.tensor_tensor(
                                minpos[:], posms[m][:, 0:1], posms[m][:, 1:2],
                                Alu.min,
                            )
                        else:
                            minpos = smp.tile([MT, 1], f32, tag="minpos")
                            nc.vector.tensor_reduce(
                                minpos[:], posms[m][:],
                                axis=mybir.AxisListType.X, op=Alu.min,
                            )
                        # t_p = min(minpos - POIS, 1)
                        nc.vector.tensor_scalar(
                            outb[:, m : m + 1], minpos, -POIS, 1.0,
                            Alu.add, Alu.min,
                        )
                        nc.vector.tensor_scalar_mul(
                            ntpall[:, m : m + 1], outb[:, m : m + 1], -1.0
                        )
                bi = hi

            # ---------------- main loop over M-tiles ----------------
            mcolof = {}
            bi = 0
            for m in range(NMT):
                for j, d in enumerate(diag[m]):
                    mcolof[(m, d)] = bi
                    bi += 1
            for m in range(NMT):
                dts = diag[m]
                lhsT = xbn[:, m * MT : (m + 1) * MT]
                ntp = ntpall[:, m : m + 1]
                u = upool.tile([MT, B], bf16, tag="u")

                for g in range(N_NT // GW):
                    wg = psw.tile([MT, GW * NT], f32, tag="w")
                    for k in range(GW):
                        t = GW * g + k
                        nc.tensor.matmul(
                            wg[:, k * NT : (k + 1) * NT],
                            lhsT, xtn[:, t * NT : (t + 1) * NT],
                        )
                    # u = 1/(dot - t_p); diag banks read wm (dot+pois) instead.
                    # The very last bank goes through DVE (add + reciprocal)
                    # to offload the Act bottleneck.
                    dve_t = -1  # DVE recip offload disabled (DVE-bound)
                    k = 0
                    while k < GW:
                        t = GW * g + k
                        if t in dts:
                            # full-bank poisoned copy, recomputed here (DVE
                            # has steady-state slack; keeps it out of the
                            # serial prologue head)
                            i = mcolof[(m, t)]
                            wmf = wmp.tile([MT, NT], f32, tag="wmf", bufs=4)
                            nc.vector.tensor_tensor(
                                wmf[:], wg[:, k * NT : (k + 1) * NT],
                                mk[:, i * NT : (i + 1) * NT], Alu.add,
                            )
                            _raw_recip_bias(
                                nc, u[:, t * NT : (t + 1) * NT], wmf[:], ntp,
                            )
                            k += 1
                        elif t == dve_t:
                            tmp = wmp.tile([MT, NT], f32, tag="tmp15", bufs=3)
                            nc.vector.tensor_scalar(
                                tmp[:], wg[:, k * NT : (k + 1) * NT],
                                ntp, None, Alu.add,
                            )
                            with nc.allow_low_precision(reason="u is bf16 by design"):
                                nc.vector.reciprocal(
                                    u[:, t * NT : (t + 1) * NT], tmp[:]
                                )
                            k += 1
                        else:
                            k2 = k
                            while (
                                k2 < GW
                                and (GW * g + k2) not in dts
                                and (GW * g + k2) != dve_t
                            ):
                                k2 += 1
                            usl = slice((GW * g + k) * NT, (GW * g + k2) * NT)
                            _raw_recip_bias(
                                nc, u[:, usl], wg[:, k * NT : k2 * NT], ntp
                            )
                            k = k2

                # r1 = min(u): tree on DVE (lags one tile behind Act)
                H = B // 2
                lh = scr.tile([MT, H // 2], bf16, tag="lh")
                nc.vector.tensor_tensor(lh[:], u[:, : H // 2], u[:, H // 2 : H], Alu.min)
                rh = scr.tile([MT, H // 2], bf16, tag="rh")
                nc.vector.tensor_tensor(
                    rh[:], u[:, H : H + H // 2], u[:, H + H // 2 :], Alu.min
                )
                cmb = scr.tile([MT, H // 2], bf16, tag="cmb")
                nc.vector.tensor_tensor(cmb[:], lh[:], rh[:], Alu.min)
                cm2 = scr.tile([MT, H // 4], bf16, tag="cm2")
                nc.vector.tensor_tensor(
                    cm2[:], cmb[:, : H // 4], cmb[:, H // 4 :], Alu.min
                )
                cm3 = scr.tile([MT, H // 8], bf16, tag="cm3")
                nc.vector.tensor_tensor(
                    cm3[:], cm2[:, : H // 8], cm2[:, H // 8 :], Alu.min
                )
                nc.vector.tensor_reduce(
                    outb[:, NMT + m : NMT + m + 1], cm3[:],
                    axis=mybir.AxisListType.X, op=Alu.min,
                )

            nc.sync.dma_start(out_d, outb[:])

    nc.compile()
    return nc


# --------------------------------------------------------------------------
# entry point
# --------------------------------------------------------------------------
def _prepare(embeddings, labels):
    emb = np.asarray(embeddings, dtype=np.float32)
    lab = np.asarray(labels).astype(np.int64)
    plan = _plan(lab)
    emb_sorted = emb[plan["order"]]
    cores = [_build_core_inputs(emb_sorted, plan, c) for c in range(NCORES)]
    mask_k = cores[0][2].shape[1]
    return emb, lab, plan, cores, mask_k


def _host_reduce(emb, lab, plan, outs):
    """outs: per core {"out": [128, 16] f32} (cols 0-7 t_p, 8-15 r1)."""
    order = plan["order"]
    slab = lab[order]
    rows_per_core = B // NCORES

    t_p = np.zeros(B, np.float64)
    r1 = np.zeros(B, np.float64)
    for c in range(NCORES):
        o = np.asarray(outs[c]["out"], np.float64)
        for m in range(NMT):
            rr = slice(c * rows_per_core + m * MT, c * rows_per_core + (m + 1) * MT)
            t_p[rr] = o[:, m]
            r1[rr] = o[:, NMT + m]

    with np.errstate(divide="ignore", invalid="ignore"):
        q = t_p + 1.0 / r1
    d_ap = 1.0 - t_p
    d_semi = 1.0 - q
    lo = t_p - MARGIN

    # validity from class counts
    _, inv, counts = np.unique(slab, return_inverse=True, return_counts=True)
    cnt_row = counts[inv]
    valid = (cnt_row >= 2) & (cnt_row <= B - 1)

    # rows needing exact recompute: no semi-hard in window, or near the
    # window boundary, or degenerate r1
    EDGE = 1e-3
    semi_ok = (q > lo + EDGE) & (q < t_p) & np.isfinite(q) & (r1 < 0)
    redo = valid & ~semi_ok

    per_row = np.where(valid, np.maximum(d_ap - d_semi + MARGIN, 0.0), 0.0)

    if redo.any():
        e = emb / np.maximum(
            np.linalg.norm(emb, axis=1, keepdims=True), 1e-12
        )
        idx = order[np.flatnonzero(redo)]  # original row indices
        for g, i in zip(np.flatnonzero(redo), idx):
            dot = (e[i] @ e.T).astype(np.float32)
            dist = np.clip(1.0 - dot, 0.0, None)
            pos = (lab == lab[i])
            pos[i] = False
            neg = lab != lab[i]
            dap = dist[pos].max()
            semi = neg & (dist > dap) & (dist < dap + MARGIN)
            if semi.any():
                dan = dist[semi].min()
            else:
                dan = dist[neg].min()
            per_row[g] = max(dap - dan + MARGIN, 0.0)

    num_valid = max(int(valid.sum()), 1)
    loss = per_row[valid].sum() / num_valid
    return np.array(loss, dtype=np.float32)


def kernel_run(embeddings, labels, trace=False):
    import concourse.bass_utils as bass_utils

    emb, lab, plan, cores, mask_k = _prepare(embeddings, labels)
    diag = plan["diag"]
    wins = plan["wins"]
    key = (
        tuple(tuple(d) for d in diag),
        tuple(tuple(w) for w in wins),
        mask_k,
    )
    if key not in _CACHE:
        _CACHE[key] = _build_bass(diag, wins, mask_k)
    nc = _CACHE[key]
    in_maps = [
        {"xt": np.ascontiguousarray(c[0]), "xb": np.ascontiguousarray(c[1]),
         "mk": np.ascontiguousarray(c[2]), "oh": np.ascontiguousarray(c[3]),
         "ob": np.ascontiguousarray(c[4])}
        for c in cores
    ]
    res = bass_utils.run_bass_kernel_spmd(
        nc, in_maps, core_ids=list(range(NCORES)), trace=trace
    )
    loss = _host_reduce(emb, lab, plan, res.results)
    return loss, res


def kernel(embeddings, labels):
    loss, _ = kernel_run(embeddings, labels)
    return loss



# revision 5
# speedup vs baseline: 1.0081x; 1.0081x over previous
"""Batch semi-hard triplet loss (cosine distance) on 8 Trainium2 NeuronCores.

Strategy (data-parallel over rows, per sharding hint):
  - Host: sort rows by label; core c takes sorted rows [1024c, 1024(c+1)) in
    8 exact 128-row M-tiles; columns rotated per core so its rows' class
    columns sit in the first PSUM group of each M-tile.
  - Device (per core, uniform SPMD program):
      * normalize embeddings (squares, one-hot column-sum matmuls, sqrt,
        reciprocal, one-hot broadcast matmuls, column scale);
      * per M-tile m: 16 matmuls (4-bank PSUM groups). Class-column poison
        (-2) is applied ON THE PE via small rank-per-class accumulate
        matmuls (lhsT = -2*row-indicators, rhs = col-indicators), so the
        diag group needs no mask adds. t_p (min positive-class dot) comes
        from a narrow poisoned window min on the first group.
        Then three engines split the threshold-max reduction
        q = max{dot < t_p}:
          - Act banks: u = 1/(dot - t_p) (Reciprocal w/ per-partition
            bias), bf16; float min-tree over u -> r1 (min u).
          - Pool banks: y = (dot min t_p) - t_p (one fused gpsimd
            tensor_scalar), bf16: candidates are negative, others +0;
            signed-int16 bit-pattern min over y picks the largest dot
            strictly below t_p (sign bit wraps the threshold).
          - DVE: runs both min-trees (u float-min, y int16-bits-min),
            one M-tile behind.
  - Host: q = max(t_p + 1/r1, t_p + y); per-row loss epilogue in f64;
    rows with no semi-hard candidate in the margin window (or near the
    branch boundary) are recomputed exactly in f32 numpy; mean over valid.
"""

import numpy as np
import ml_dtypes

B = 8192
D = 128
MARGIN = 0.2
NCORES = 8
NT = 512            # N-tile width (one PSUM bank of fp32)
N_NT = B // NT      # 16
MT = 128            # M-tile rows
NMT = B // NCORES // MT  # 8 m-tiles per core
GW = 4              # N-tiles per PSUM group tile
POIS = -2.0         # class-column poison (exactly representable in bf16)

# bank families: Act does recip on banks 2..11; Pool shifts banks 0,1,12..15
ACT_BANKS = list(range(2, 12))
POOL_BANKS = [0, 1, 12, 13, 14, 15]
NA = len(ACT_BANKS)
NP_ = len(POOL_BANKS)

BF16 = ml_dtypes.bfloat16

_CACHE = {}


# --------------------------------------------------------------------------
# host-side planning (pure layout, computed from labels)
# --------------------------------------------------------------------------
def _plan(labels: np.ndarray):
    order = np.argsort(labels, kind="stable")
    slab = labels[order]
    bounds = np.flatnonzero(np.r_[True, slab[1:] != slab[:-1], True])
    cls_start, cls_end = bounds[:-1], bounds[1:]
    row_s = np.empty(B, dtype=np.int64)
    row_e = np.empty(B, dtype=np.int64)
    for s, e in zip(cls_start, cls_end):
        row_s[s:e] = s
        row_e[s:e] = e

    rows_per_core = B // NCORES
    cores = []
    for c in range(NCORES):
        r0 = c * rows_per_core
        base = int(row_s[r0])  # start of first class -> no wraparound
        diag = []
        for m in range(NMT):
            rr = slice(r0 + m * MT, r0 + (m + 1) * MT)
            s = row_s[rr] - base
            e = row_e[rr] - base
            dts = sorted(set((s // NT).tolist()) | set(((e - 1) // NT).tolist()))
            diag.append(dts)
        cores.append(dict(r0=r0, base=base, diag=diag))
    # unify diag sets across cores so all 8 run one compiled program
    uni = [
        sorted(set().union(*[set(pc["diag"][m]) for pc in cores]))
        for m in range(NMT)
    ]
    for pc in cores:
        pc["diag"] = uni
    # per (m, diag tile): narrow column window [c0, c1) within the bank that
    # contains every class column of the tile's rows, across all cores
    wins = []
    for m in range(NMT):
        wm_ = []
        for d in uni[m]:
            c0, c1 = NT, 0
            for pc in cores:
                rr = slice(pc["r0"] + m * MT, pc["r0"] + (m + 1) * MT)
                s = np.maximum(row_s[rr] - pc["base"] - d * NT, 0)
                e = np.minimum(row_e[rr] - pc["base"] - d * NT, NT)
                ok = s < e
                if ok.any():
                    c0 = min(c0, int(s[ok].min()))
                    c1 = max(c1, int(e[ok].max()))
            if c1 <= c0:
                c0, c1 = 0, NT
            wm_.append((c0, c1))
        wins.append(wm_)
    # max classes per (m, diag-tile) block across cores (pois matmul k-dim)
    cp = 1
    for c in range(NCORES):
        pc = cores[c]
        r0, base = pc["r0"], pc["base"]
        for m in range(NMT):
            rr = slice(r0 + m * MT, r0 + (m + 1) * MT)
            ss = row_s[rr]
            for d in uni[m]:
                lo, hi = base + d * NT, base + (d + 1) * NT
                # classes whose column range intersects the bank
                cls = set()
                for g in range(rr.start, rr.stop):
                    if row_s[g] < hi and row_e[g] > lo:
                        cls.add(int(row_s[g]))
                cp = max(cp, len(cls))
    return dict(
        order=order, row_s=row_s, row_e=row_e, cores=cores, diag=uni,
        wins=wins, cp=cp,
    )


def _build_core_inputs(emb_sorted: np.ndarray, plan, c: int):
    """Returns (xt_rot [D,B], xb [D,1024], pl [CP, nblk*MT], pr [CP, nblk*NT],
    oh [D, 4*NOH], ob [4, D*NOH]) all bf16."""
    pc = plan["cores"][c]
    base, r0 = pc["base"], pc["r0"]
    rows_per_core = B // NCORES
    row_s, row_e = plan["row_s"], plan["row_e"]
    cp = plan["cp"]

    rot = np.r_[np.arange(base, B), np.arange(0, base)]
    xt_rot = np.ascontiguousarray(emb_sorted[rot].T).astype(BF16)
    xb = np.ascontiguousarray(emb_sorted[r0 : r0 + rows_per_core].T).astype(BF16)

    # poison matmul blocks: per (m, d in diag[m]):
    #   pl[k, i] = -2 if m-tile row i in class k else 0     [CP, MT]
    #   pr[k, j] = 1 if bank-d col j in class k else 0      [CP, NT]
    nblk = sum(len(d) for d in pc["diag"])
    pl = np.zeros((cp, nblk * MT), np.float32)
    pr = np.zeros((cp, nblk * NT), np.float32)
    bi = 0
    for m in range(NMT):
        for d in pc["diag"][m]:
            lo, hi = base + d * NT, base + (d + 1) * NT
            cls = {}
            for r in range(MT):
                g = r0 + m * MT + r
                s, e = int(row_s[g]), int(row_e[g])
                if s < hi and e > lo:
                    k = cls.setdefault(s, len(cls))
                    pl[k, bi * MT + r] = POIS
                    cs, ce = max(s - lo, 0), min(e - lo, NT)
                    pr[k, bi * NT + cs : bi * NT + ce] = 1.0
            assert len(cls) <= cp
            bi += 1
    pl = pl.astype(BF16)
    pr = pr.astype(BF16)

    # one-hot helper blocks for the normalize matmuls
    NOH = N_NT + 2
    oh = np.zeros((D, 4 * NOH), np.float32)
    for t in range(NOH):
        oh[:, 4 * t + (t % 4)] = 1.0
    oh = oh.astype(BF16)
    ob = np.zeros((4, D * NOH), np.float32)
    for t in range(NOH):
        ob[t % 4, D * t : D * (t + 1)] = 1.0
    ob = ob.astype(BF16)
    return xt_rot, xb, pl, pr, oh, ob


# --------------------------------------------------------------------------
# device program
# --------------------------------------------------------------------------
def _raw_recip_bias(nc, out, in_, bias_ap):
    import concourse.mybir as mybir

    eng = nc.scalar
    ins = [
        eng.lower_ap(in_),
        eng.lower_ap(bias_ap),
        mybir.ImmediateValue(dtype=mybir.dt.float32, value=1.0),  # scale
        mybir.ImmediateValue(dtype=mybir.dt.float32, value=0.0),  # alpha
    ]
    return eng.add_instruction(
        mybir.InstActivation(
            name=f"I-{nc.next_id()}",
            func=mybir.ActivationFunctionType.Reciprocal,
            ins=ins,
            outs=[eng.lower_ap(out)],
        )
    )


def _build_bass(diag, wins, cp):
    import concourse.bacc as bacc
    import concourse.mybir as mybir
    from concourse.tile import TileContext

    f32 = mybir.dt.float32
    bf16 = mybir.dt.bfloat16
    i16 = mybir.dt.int16
    Alu = mybir.AluOpType
    Act = mybir.ActivationFunctionType
    NOH = N_NT + 2
    NBC = NMT * MT  # xb columns (1024)
    nblk = sum(len(d) for d in diag)

    nc = bacc.Bacc("TRN2", target_bir_lowering=False, debug=False, num_devices=NCORES)

    xt_d = nc.dram_tensor("xt", [D, B], bf16, kind="ExternalInput").ap()
    xb_d = nc.dram_tensor("xb", [D, NBC], bf16, kind="ExternalInput").ap()
    pl_d = nc.dram_tensor("pl", [cp, nblk * MT], bf16, kind="ExternalInput").ap()
    pr_d = nc.dram_tensor("pr", [cp, nblk * NT], bf16, kind="ExternalInput").ap()
    oh_d = nc.dram_tensor("oh", [D, 4 * NOH], bf16, kind="ExternalInput").ap()
    ob_d = nc.dram_tensor("ob", [4, D * NOH], bf16, kind="ExternalInput").ap()
    out_d = nc.dram_tensor("out", [MT, 2 * NMT], f32, kind="ExternalOutput").ap()
    outy_d = nc.dram_tensor("outy", [MT, NMT], bf16, kind="ExternalOutput").ap()

    # diag-block flat index per (m, d)
    blkof = {}
    bi = 0
    for m in range(NMT):
        for j, d in enumerate(diag[m]):
            blkof[(m, d)] = bi
            bi += 1

    with TileContext(nc) as tc:
        with (
            tc.tile_pool(name="big", bufs=1) as big,
            tc.tile_pool(name="upool", bufs=2) as upool,
            tc.tile_pool(name="ypool", bufs=2) as ypool,
            tc.tile_pool(name="scr", bufs=2) as scr,
            tc.tile_pool(name="sm", bufs=6) as smp,
            tc.tile_pool(name="psw", bufs=2, space="PSUM") as psw,
        ):
            # ---------------- setup: load + normalize (pipelined) -----------
            oh = big.tile([D, 4 * NOH], bf16, tag="oh")
            nc.sync.dma_start(oh[:], oh_d)
            ob = big.tile([4, D * NOH], bf16, tag="ob")
            nc.sync.dma_start(ob[:], ob_d)
            xb = big.tile([D, NBC], bf16, tag="xb")
            nc.sync.dma_start(xb[:], xb_d)
            pl = big.tile([cp, nblk * MT], bf16, tag="pl")
            nc.sync.dma_start(pl[:], pl_d)
            pr = big.tile([cp, nblk * NT], bf16, tag="pr")
            nc.sync.dma_start(pr[:], pr_d)
            xt = big.tile([D, B], bf16, tag="xt")
            for j in range(8):
                sl = slice(j * (B // 8), (j + 1) * (B // 8))
                nc.sync.dma_start(xt[:, sl], xt_d[:, sl])

            sq = big.tile([D, NOH * NT], bf16, tag="sq")
            xtn = big.tile([D, B], bf16, tag="xtn")
            xbn = big.tile([D, NBC], bf16, tag="xbn")
            outb = big.tile([MT, 2 * NMT], f32, tag="outb")
            outy = big.tile([MT, NMT], bf16, tag="outy")
            ntpall = big.tile([MT, NMT], f32, tag="ntpall")

            def norm_stage_a(nrows, tbase, src, act_sq):
                """squares + one-hot n2 matmuls + psum->sbuf copy + sqrt"""
                w = nrows * NT
                if act_sq:
                    nc.scalar.activation(sq[:, tbase * NT : tbase * NT + w],
                                         src, Act.Square)
                else:
                    nc.vector.tensor_tensor(sq[:, tbase * NT : tbase * NT + w],
                                            src, src, Alu.mult)
                pn = psw.tile([MT, GW * NT], f32, tag="w", name="pn")
                for k in range(nrows):
                    t = tbase + k
                    nc.tensor.matmul(
                        pn[0:4, :NT], oh[:, 4 * t : 4 * (t + 1)],
                        sq[:, t * NT : (t + 1) * NT],
                        start=(k == 0), stop=(k == nrows - 1),
                    )
                n2g = smp.tile([4, NT], f32, tag="n2g", name="n2g", bufs=3)
                nc.scalar.copy(n2g[0:nrows, :], pn[0:nrows, :NT])
                s0g = smp.tile([4, NT], f32, tag="s0g", name="s0g", bufs=3)
                nc.scalar.activation(s0g[0:nrows, :], n2g[0:nrows, :], Act.Sqrt)
                return s0g

            def norm_stage_b(s0g, nrows, tbase, src, dst):
                """reciprocal -> bf16 rn -> broadcast matmuls -> scaled dst"""
                w = nrows * NT
                r0g = smp.tile([4, NT], f32, tag="r0g", name="r0g", bufs=3)
                nc.vector.reciprocal(r0g[0:nrows, :], s0g[0:nrows, :])
                rng_ = smp.tile([4, NT], bf16, tag="rng", name="rng", bufs=3)
                nc.scalar.copy(rng_[0:nrows, :], r0g[0:nrows, :])
                gp = psw.tile([MT, GW * NT], f32, tag="w", name="gp")
                for k in range(nrows):
                    t = tbase + k
                    nc.tensor.matmul(
                        gp[:, k * NT : (k + 1) * NT],
                        ob[0:nrows, D * t : D * (t + 1)], rng_[0:nrows, :],
                    )
                nc.vector.tensor_tensor(dst, src, gp[:, :w], Alu.mult)

            # two-stage pipelined chains: xb first (xbn gates the main loop)
            chains = [
                (2, N_NT, xb[:], xbn[:], False),
            ] + [
                (4, 4 * gq, xt[:, 4 * gq * NT : (4 * gq + 4) * NT],
                 xtn[:, 4 * gq * NT : (4 * gq + 4) * NT], gq >= 2)
                for gq in range(4)
            ]
            pend = None
            for ch in chains:
                nrows, tbase, src, dst, act_sq = ch
                s0g = norm_stage_a(nrows, tbase, src, act_sq)
                if pend is not None:
                    norm_stage_b(*pend)
                pend = (s0g, nrows, tbase, src, dst)
            norm_stage_b(*pend)

            # ---------------- main loop over M-tiles ----------------
            # per-bank slot in the u (Act) / y (Pool) buffers
            uslot = {b: i for i, b in enumerate(ACT_BANKS)}
            yslot = {b: i for i, b in enumerate(POOL_BANKS)}

            pending_tree = None  # (u, y, m) of previous M-tile

            def emit_trees(u, y, m):
                # u float min-tree: [MT, NA*NT] bf16 -> r1 -> outb[:, NMT+m]
                w = NA * NT  # 5120
                t1 = scr.tile([MT, w // 2], bf16, tag="ut1")
                nc.vector.tensor_tensor(t1[:], u[:, : w // 2], u[:, w // 2 :], Alu.min)
                t2 = scr.tile([MT, w // 4], bf16, tag="ut2")
                nc.vector.tensor_tensor(
                    t2[:], t1[:, : w // 4], t1[:, w // 4 :], Alu.min
                )
                t3 = scr.tile([MT, w // 8], bf16, tag="ut3")
                nc.vector.tensor_tensor(
                    t3[:], t2[:, : w // 8], t2[:, w // 8 :], Alu.min
                )
                t4 = scr.tile([MT, w // 16], bf16, tag="ut4")
                nc.vector.tensor_tensor(
                    t4[:], t3[:, : w // 16], t3[:, w // 16 :], Alu.min
                )
                t5 = scr.tile([MT, w // 32], bf16, tag="ut5")
                nc.vector.tensor_tensor(
                    t5[:], t4[:, : w // 32], t4[:, w // 32 :], Alu.min
                )
                nc.vector.tensor_reduce(
                    outb[:, NMT + m : NMT + m + 1], t5[:],
                    axis=mybir.AxisListType.X, op=Alu.min,
                )
                # y int16-bits min-tree: [MT, NP_*NT] bf16 -> outy[:, m]
                wy = NP_ * NT  # 3072
                yi = y[:].bitcast(i16)
                s1 = scr.tile([MT, wy // 2], i16, tag="yt1")
                nc.vector.tensor_tensor(
                    s1[:], yi[:, : wy // 2], yi[:, wy // 2 :], Alu.min
                )
                s2 = scr.tile([MT, wy // 4], i16, tag="yt2")
                nc.vector.tensor_tensor(
                    s2[:], s1[:, : wy // 4], s1[:, wy // 4 :], Alu.min
                )
                s3 = scr.tile([MT, wy // 8], i16, tag="yt3")
                nc.vector.tensor_tensor(
                    s3[:], s2[:, : wy // 8], s2[:, wy // 8 :], Alu.min
                )
                s4 = scr.tile([MT, wy // 16], i16, tag="yt4")
                nc.vector.tensor_tensor(
                    s4[:], s3[:, : wy // 16], s3[:, wy // 16 :], Alu.min
                )
                nc.vector.tensor_reduce(
                    outy[:, m : m + 1].bitcast(i16), s4[:],
                    axis=mybir.AxisListType.X, op=Alu.min,
                )

            for m in range(NMT):
                dts = diag[m]
                lhsT = xbn[:, m * MT : (m + 1) * MT]
                tpp = outb[:, m : m + 1]       # +t_p ptr (f32)
                ntp = ntpall[:, m : m + 1]     # -t_p ptr (f32)
                u = upool.tile([MT, NA * NT], bf16, tag="u")
                y = ypool.tile([MT, NP_ * NT], bf16, tag="y")

                need_groups = sorted({d // GW for d in dts})
                group_order = need_groups + [
                    g for g in range(N_NT // GW) if g not in need_groups
                ]
                wgs = {}
                deferred = []  # consumer groups awaiting t_p
                for gi, g in enumerate(group_order):
                    wg = psw.tile([MT, GW * NT], f32, tag="w")
                    wgs[g] = wg
                    for k in range(GW):
                        t = GW * g + k
                        if t in dts:
                            nc.tensor.matmul(
                                wg[:, k * NT : (k + 1) * NT],
                                lhsT, xtn[:, t * NT : (t + 1) * NT],
                                start=True, stop=False,
                            )
                            i = blkof[(m, t)]
                            nc.tensor.matmul(
                                wg[:, k * NT : (k + 1) * NT],
                                pl[:, i * MT : (i + 1) * MT],
                                pr[:, i * NT : (i + 1) * NT],
                                start=False, stop=True,
                            )
                        else:
                            nc.tensor.matmul(
                                wg[:, k * NT : (k + 1) * NT],
                                lhsT, xtn[:, t * NT : (t + 1) * NT],
                            )

                    tp_ready = gi >= len(need_groups) - 1
                    if gi == len(need_groups) - 1:
                        # t_p chain: narrow poisoned window mins -> t_p, -t_p
                        ndts = len(dts)
                        posm = smp.tile([MT, max(ndts, 1)], f32, tag="posm")
                        for j, d in enumerate(dts):
                            c0, c1 = wins[m][j]
                            wgd = wgs[d // GW]
                            k = d % GW
                            nc.vector.tensor_reduce(
                                posm[:, j : j + 1],
                                wgd[:, k * NT + c0 : k * NT + c1],
                                axis=mybir.AxisListType.X, op=Alu.min,
                            )
                        if ndts == 1:
                            minpos = posm[:, 0:1]
                        elif ndts == 2:
                            mp = smp.tile([MT, 1], f32, tag="minpos")
                            nc.vector.tensor_tensor(
                                mp[:], posm[:, 0:1], posm[:, 1:2], Alu.min
                            )
                            minpos = mp[:]
                        else:
                            mp = smp.tile([MT, 1], f32, tag="minpos")
                            nc.vector.tensor_reduce(
                                mp[:], posm[:], axis=mybir.AxisListType.X,
                                op=Alu.min,
                            )
                            minpos = mp[:]
                        # t_p = min(minpos - POIS, 1)
                        nc.vector.tensor_scalar(
                            tpp, minpos, -POIS, 1.0, Alu.add, Alu.min
                        )
                        nc.vector.tensor_scalar_mul(ntp, tpp, -1.0)
                        # previous M-tile's trees go after the t_p chain so
                        # t_p never queues behind heavy DVE work
                        if pending_tree is not None:
                            emit_trees(*pending_tree)
                            pending_tree = None

                    def consume(g, wg):
                        bank0 = GW * g
                        # contiguous Act banks in this group
                        ab = [b for b in range(bank0, bank0 + GW) if b in uslot]
                        if ab:
                            k0, k1 = ab[0] - bank0, ab[-1] - bank0 + 1
                            _raw_recip_bias(
                                nc,
                                u[:, uslot[ab[0]] * NT : uslot[ab[-1]] * NT + NT],
                                wg[:, k0 * NT : k1 * NT],
                                ntp,
                            )
                        pb = [b for b in range(bank0, bank0 + GW) if b in yslot]
                        if pb:
                            k0, k1 = pb[0] - bank0, pb[-1] - bank0 + 1
                            nc.gpsimd.tensor_scalar(
                                y[:, yslot[pb[0]] * NT : yslot[pb[-1]] * NT + NT],
                                wg[:, k0 * NT : k1 * NT],
                                tpp, tpp, Alu.min, Alu.subtract,
                            )

                    if not tp_ready:
                        deferred.append((g, wg))
                    else:
                        for dg, dwg in deferred:
                            consume(dg, dwg)
                        deferred = []
                        consume(g, wg)

                pending_tree = (u, y, m)

            emit_trees(*pending_tree)

            nc.sync.dma_start(out_d, outb[:])
            nc.sync.dma_start(outy_d, outy[:])

    nc.compile()
    return nc


# --------------------------------------------------------------------------
# entry point
# --------------------------------------------------------------------------
def _prepare(embeddings, labels):
    emb = np.asarray(embeddings, dtype=np.float32)
    lab = np.asarray(labels).astype(np.int64)
    plan = _plan(lab)
    emb_sorted = emb[plan["order"]]
    cores = [_build_core_inputs(emb_sorted, plan, c) for c in range(NCORES)]
    return emb, lab, plan, cores


def _host_reduce(emb, lab, plan, outs):
    """outs: per core {"out": [128, 16] f32, "outy": [128, 8] bf16}."""
    order = plan["order"]
    slab = lab[order]
    rows_per_core = B // NCORES

    t_p = np.zeros(B, np.float64)
    r1 = np.zeros(B, np.float64)
    yw = np.zeros(B, np.float64)
    for c in range(NCORES):
        o = np.asarray(outs[c]["out"], np.float64)
        oy = np.asarray(outs[c]["outy"]).astype(np.float64)
        for m in range(NMT):
            rr = slice(c * rows_per_core + m * MT, c * rows_per_core + (m + 1) * MT)
            t_p[rr] = o[:, m]
            r1[rr] = o[:, NMT + m]
            yw[rr] = oy[:, m]

    with np.errstate(divide="ignore", invalid="ignore"):
        q1 = t_p + 1.0 / r1
    q2 = t_p + yw
    c1 = (r1 < 0) & np.isfinite(q1)
    c2 = yw < 0
    q = np.where(
        c1 & c2, np.maximum(q1, q2), np.where(c1, q1, np.where(c2, q2, -np.inf))
    )
    d_ap = 1.0 - t_p
    d_semi = 1.0 - q
    lo = t_p - MARGIN

    # validity from class counts
    _, inv, counts = np.unique(slab, return_inverse=True, return_counts=True)
    cnt_row = counts[inv]
    valid = (cnt_row >= 2) & (cnt_row <= B - 1)

    EDGE = 1e-3
    semi_ok = (c1 | c2) & (q > lo + EDGE) & (q < t_p) & np.isfinite(q)
    redo = valid & ~semi_ok

    per_row = np.where(valid, np.maximum(d_ap - d_semi + MARGIN, 0.0), 0.0)

    if redo.any():
        e = emb / np.maximum(
            np.linalg.norm(emb, axis=1, keepdims=True), 1e-12
        )
        idx = order[np.flatnonzero(redo)]  # original row indices
        for g, i in zip(np.flatnonzero(redo), idx):
            dot = (e[i] @ e.T).astype(np.float32)
            dist = np.clip(1.0 - dot, 0.0, None)
            pos = (lab == lab[i])
            pos[i] = False
            neg = lab != lab[i]
            dap = dist[pos].max()
            semi = neg & (dist > dap) & (dist < dap + MARGIN)
            if semi.any():
                dan = dist[semi].min()
            else:
                dan = dist[neg].min()
            per_row[g] = max(dap - dan + MARGIN, 0.0)

    num_valid = max(int(valid.sum()), 1)
    loss = per_row[valid].sum() / num_valid
    return np.array(loss, dtype=np.float32)


def kernel_run(embeddings, labels, trace=False):
    import concourse.bass_utils as bass_utils

    emb, lab, plan, cores = _prepare(embeddings, labels)
    diag = plan["diag"]
    wins = plan["wins"]
    cp = plan["cp"]
    key = (
        tuple(tuple(d) for d in diag),
        tuple(tuple(w) for w in wins),
        cp,
    )
    if key not in _CACHE:
        _CACHE[key] = _build_bass(diag, wins, cp)
    nc = _CACHE[key]
    in_maps = [
        {"xt": np.ascontiguousarray(c[0]), "xb": np.ascontiguousarray(c[1]),
         "pl": np.ascontiguousarray(c[2]), "pr": np.ascontiguousarray(c[3]),
         "oh": np.ascontiguousarray(c[4]), "ob": np.ascontiguousarray(c[5])}
        for c in cores
    ]
    res = bass_utils.run_bass_kernel_spmd(
        nc, in_maps, core_ids=list(range(NCORES)), trace=trace
    )
    loss = _host_reduce(emb, lab, plan, res.results)
    return loss, res


def kernel(embeddings, labels):
    loss, _ = kernel_run(embeddings, labels)
    return loss
